# revision 62
# baseline (speedup 1.0000x reference)
"""Llama decoder layer (S=4096, D=768, NH=12, I=3072, fp32) on 8 TRN2 cores.

Strategy: sequence-sharded data parallel with ZERO collectives. Each core
receives the full hidden_states (replicated, bf16) plus its own 512-row
shard, and redundantly computes the full K^T / V (bf16) it needs for
attention over all 4096 keys: ~120us of extra PE work instead of the
~250us serial AllGather the previous version paid, and no cross-core
synchronization at all.

Schedule (HW-measured ~0.53ms/exec via on-device repeats; the ~80ms
blocking wall time per call is pure axon-tunnel RTT, see test.py):
  - phase A: per-512-row-group software pipeline — group g+1's
    rmsnorm chain (Act square+accum_out, DVE scale) is emitted before
    group g's K/V matmuls so FIFO engine queues never park a norm
    behind PSUM-eviction waits; x loads ride the SP queue, wq/wk/wv
    the Act queue. The xn transposes are DMA transposes of the RAW
    bf16 rows on the Act queue, followed by one zero-stride-broadcast
    DVE multiply per block that applies the rsqrt column scales and
    casts to fp8 — replacing 24 PE transposes + 8 PSUM evictions per
    group with 4 DMA ops + 4 DVE ops (this alone was worth ~265us:
    the PE-transpose chain serialized phase A through shared PSUM)
  - o_proj AND all MLP weights are DMA-prefetched on the idle SP queue
    at the end of phase A: emitting those dma_starts in later phases
    would queue them behind the whole attention exp stream on the
    issuing engine, stalling the MLP ~60us on its weight feed
  - attention: scores per (head-pair, key-block) as two fp8 DoubleRow
    matmuls -> [128,1024] exp -> AV accumulated directly in PSUM
    across all 32 key blocks; softmax denominator via a ones-column in
    V. The exp stream splits 9:7 across Act (native Exp) and DVE
    (direct-to-int8 Schraudolph: the fp8e4m3 bit pattern of exp(x) ~
    int8(8*(x/ln2+7)-0.5), one tensor_scalar written through an int8
    bitcast of the fp8 eT; ~2.6% per-element error, washed out by
    softmax normalization). AV is an fp8 DoubleRow matmul over
    key-block PAIRS (V slots padded to 80B/head so every Ldweights
    stride/base is 16-aligned). Triple-buffered score PSUM, AV
    emission lagged 2 pairs, next scores hoisted before the normalize
  - o_proj + rmsnorm2 run inside the kv pool scope, overlapping the
    attention tail; rmsnorm2's transpose runs on the (now idle) SP
    DMA queue; MLP starts with all weights already resident
  - Q/K^T/V, scores, o_proj and down-proj run as fp8e4 DoubleRow
    matmuls (one instruction contracts a row-pair at 2x rate): weights
    are pre-scaled x64 host-side to escape e4m3's subnormal range, the
    1/4096 (QK both x64) folds into the exp scale, the softmax
    ones-column (V, set to 64) or a fused scalar_tensor_tensor on the
    residual add (o_proj/down); up-proj stays bf16 (the error-sensitive
    path). The scores DoubleRow runs on a head-interleaved fp8 K^T/Q^T
    layout (4 heads x 32-dim halves per partition block, halves at
    free-dim stride S/SH) produced for free by permuting wq/wk columns
    host-side. fp32 PSUM accumulation throughout; gate is 2e-2.
"""
import os
import numpy as np

_PROBE = os.environ.get("KPROBE", "")  # bisection probes; "" in production
import ml_dtypes
import concourse.bass as bass
import concourse.tile as tile
from concourse import bacc, mybir
from concourse import masks
from concourse.bass_utils import run_bass_kernel_spmd

dt = mybir.dt
AF = mybir.ActivationFunctionType

N_CORES = 8
S, D, NH, HD, I = 4096, 768, 12, 64, 3072
SH = S // N_CORES          # 512 rows per core
NSB = SH // 128            # 4 s-blocks per local shard
NG = S // SH               # 8 groups of 512 rows (full sequence)
NKT = D // 128             # 6 contraction tiles over D
NIT = I // 128             # 24 i-tiles
PAIRS = NH // 2            # 6 head pairs
NKB = S // 128             # 32 key blocks
VW = 80 * NH               # 960: V row width, 80B/head (65 used + pad)
HWID = 80                  # head slot width (16-aligned for DoubleRow)
NQB = 8                    # K^T/Q^T column groups: 4 head-triples x 2 halves
DP = NQB * 128             # padded wq/wk column count (96 real + 32 zero per group)
EPS = 1e-5
SCALE = 1.0 / np.sqrt(HD)

_CACHED = {}


def _build(reps=1):
    nc = bacc.Bacc("TRN2", target_bir_lowering=False, debug=False,
                   enable_asserts=False, num_devices=N_CORES)
    Xs = nc.dram_tensor("x_shard", [SH, D], dt.bfloat16, kind="ExternalInput")
    Xf = nc.dram_tensor("x_full", [S, D], dt.bfloat16, kind="ExternalInput")
    WQ = nc.dram_tensor("wqT", [D, DP], dt.float8e4, kind="ExternalInput")
    WK = nc.dram_tensor("wkT", [D, DP], dt.float8e4, kind="ExternalInput")
    WV = nc.dram_tensor("wvT", [D, D], dt.float8e4, kind="ExternalInput")
    WO = nc.dram_tensor("woT", [D, D], dt.float8e4, kind="ExternalInput")
    WU = nc.dram_tensor("wupT", [D, I], dt.bfloat16, kind="ExternalInput")
    WD = nc.dram_tensor("wdownT", [I, D], dt.float8e4, kind="ExternalInput")
    OUT = nc.dram_tensor("out", [SH, D], dt.float32, kind="ExternalOutput")

    with tile.TileContext(nc) as tc:
        for _rep in range(reps):
            _emit_body(nc, tc, Xs, Xf, WQ, WK, WV, WO, WU, WD, OUT)
    nc.compile()
    return nc


def _emit_body(nc, tc, Xs, Xf, WQ, WK, WV, WO, WU, WD, OUT):
    if True:
        with tc.tile_pool(name="glob", bufs=1) as glob, \
             tc.tile_pool(name="tailp", bufs=1) as tailp:
            ident = glob.tile([128, 128], dt.bfloat16)
            masks.make_identity(nc, ident[:])
            x_res = glob.tile([128, NSB * D], dt.bfloat16)
            oT = glob.tile([128, PAIRS * SH], dt.float8e4)
            ssum = glob.tile([128, NG * 4 + 4], dt.float32)
            rrs = glob.tile([128, NG * 4 + 4], dt.float32)

            # tail pool: weights needed after the kv pool closes. All are
            # DMA-loaded early on the (otherwise idle) SP queue at the end
            # of phase A, so o_proj and the MLP never wait on weight DMAs
            # (emitting them in phase C/E would queue the dma_starts behind
            # the whole attention exp stream on the issuing engine).
            wo_all = tailp.tile([128, NKT * D], dt.float8e4)
            wup_all = tailp.tile([128, NKT * I], dt.bfloat16)
            wd_all = tailp.tile([128, NIT * D], dt.float8e4)

            def sq_accum(workp, xblk, scol):
                """rmsnorm sum-of-squares of one [128, D] block on Act."""
                dump = workp.tile([128, D], dt.bfloat16, tag="sqdump", name="dump")
                nc.scalar.activation(dump[:], xblk, AF.Square,
                                     accum_out=ssum[:, scol:scol + 1])

            def rr_for(cols0, ncols, workp):
                mv = workp.tile([128, 4], dt.float32, tag="mv", name="mv")
                nc.vector.tensor_scalar(out=mv[:, 0:ncols],
                                        in0=ssum[:, cols0:cols0 + ncols],
                                        scalar1=1.0 / D, scalar2=EPS,
                                        op0=mybir.AluOpType.mult,
                                        op1=mybir.AluOpType.add)
                rv = workp.tile([128, 4], dt.float32, tag="rv", name="rv")
                nc.vector.reciprocal(rv[:, 0:ncols], mv[:, 0:ncols])
                nc.scalar.activation(rrs[:, cols0:cols0 + ncols],
                                     rv[:, 0:ncols], AF.Sqrt)

            with tc.tile_pool(name="kvp", bufs=1) as kvp:
                # K^T / Q^T live in fp8 in a head-interleaved layout so the
                # scores matmuls can run as fp8 DoubleRow (0.5 cyc/row):
                # column-group g = (tri, half) with tri = head-triple, half
                # = upper/lower 32 of the 64-dim head; partitions = 3 heads
                # x 32 dims at bases {0,32,64} (the only legal matmul
                # bases). A head's two halves sit at free-dim stride S (kT)
                # / SH (qT), giving the [32, 2, cols] DoubleRow APs. The
                # layout is produced for free by permuting (and zero-
                # padding) wq/wk columns host-side (see _prep_in_maps).
                kT = kvp.tile([128, NQB * S], dt.float8e4)
                v_full = kvp.tile([128, NKB * VW], dt.float8e4)
                qT = kvp.tile([128, NQB * SH], dt.float8e4)

                # ======= phase A: norms + Q/K/V (fused pipeline) =======
                # Three DMA streams: x loads on gpsimd/SWDGE, weight loads
                # on the Act HWDGE queue, transposes on the SP queue.
                with tc.tile_pool(name="a2", bufs=1) as a2, \
                     tc.tile_pool(name="a2x", bufs=3) as a2x, \
                     tc.tile_pool(name="a2w", bufs=2) as a2w, \
                     tc.tile_pool(name="ps_k", bufs=3, space="PSUM") as ps_k, \
                     tc.tile_pool(name="ps_v", bufs=3, space="PSUM") as ps_v:
                    xgs = {}

                    def emit_xg_dma(g):
                        xg = a2x.tile([128, 4 * D], dt.bfloat16, tag="xg", name="xg")
                        # groups 0/1 stay on SP (head of the queue, nothing
                        # ahead of them); later groups alternate SP/Act so
                        # the 6MB of x traffic splits across both HWDGE
                        # queues alongside the alternating transposes
                        eng = nc.sync if (g < 2 or g % 2 == 0) else nc.scalar
                        eng.dma_start(
                            xg[:].rearrange("p (b c) -> p b c", b=4),
                            Xf.ap()[g * SH:(g + 1) * SH, :].rearrange("(b p) c -> p b c", p=128))
                        xgs[g] = xg

                    # x loads first so nothing delays group 0's chain
                    emit_xg_dma(0)
                    emit_xg_dma(1)
                    # ones columns of V (softmax denominator trick)
                    # V carries a x64 weight scale (fp8 subnormal avoidance);
                    # the ones column matches so o/Z is scale-invariant
                    nc.gpsimd.memset(
                        v_full[:].rearrange("p (kb h c) -> p kb h c", h=NH, c=HWID)[:, :, :, 64:65],
                        64.0)
                    nc.gpsimd.memset(
                        v_full[:].rearrange("p (kb h c) -> p kb h c", h=NH, c=HWID)[:, :, :, 65:HWID],
                        0.0)
                    wq_all = a2.tile([128, NKT * DP], dt.float8e4)
                    wk_all = a2.tile([128, NKT * DP], dt.float8e4)
                    wv_all = a2.tile([128, NKT * D], dt.float8e4)

                    def emit_w_dma(w_t, w_d):
                        # q/k/v weight loads ride the SWDGE queue, which is
                        # otherwise idle until the deferred o/MLP prefetch —
                        # keeping both HWDGE queues free for x loads and
                        # transposes
                        nc.gpsimd.dma_start(
                            w_t[:].rearrange("p (kt c) -> p kt c", kt=NKT),
                            w_d.ap().rearrange("(kt p) c -> p kt c", p=128))

                    emit_w_dma(wk_all, WK)

                    def norm_group(src, scol0, pe_transpose=False, out_dtype=dt.bfloat16):
                        """rmsnorm 4 blocks of [128, D] from src, return the
                        transposed [128, NKT*SH] tile (out_dtype).

                        Transpose-then-scale: the RAW bf16 rows are DMA-
                        transposed (1 instruction per block instead of 6 PE
                        transposes + 2 PSUM evictions), then the per-row
                        rsqrt factors — gathered into a [1,128] row by a
                        tiny partition-gather DMA and partition_broadcast —
                        scale the transposed COLUMNS in one DVE multiply per
                        block with a zero-stride kt-broadcast AP, casting
                        straight to fp8. ~20 fewer instructions per group
                        and no PE involvement."""
                        for b in range(4):
                            sq_accum(a2w, src[:, b * D:(b + 1) * D], scol0 + b)
                        rr_for(scol0, 4, a2w)
                        xnT_g = a2x.tile([128, NKT * SH], out_dtype, tag="xnT", name="xnT_g")
                        xT_g = a2x.tile([128, NKT * SH], dt.bfloat16, tag="xT",
                                        name="xT_g", bufs=2)
                        xT3 = xT_g[:].rearrange("p (kt m) -> p kt m", kt=NKT)
                        xnT3 = xnT_g[:].rearrange("p (kt m) -> p kt m", kt=NKT)
                        for b in range(4):
                            # alternate the two HWDGE queues: balances ~7MB
                            # of transpose traffic against the x loads (SP)
                            # and weight loads (Act), and lets a group's
                            # transposes run on both queues in parallel
                            dma_eng = nc.scalar if b % 2 == 0 else nc.sync
                            dma_eng.dma_start(
                                xT3[:, :, b * 128:(b + 1) * 128],
                                src[:, b * D:(b + 1) * D],
                                transpose=True)
                            rrow = a2w.tile([1, 128], dt.float32, tag="rrow", name="rrow")
                            nc.gpsimd.dma_start(rrow[:], rrs[:, scol0 + b:scol0 + b + 1])
                            zbc = a2w.tile([128, 128], dt.float32, tag="zbc2", name="zbc2")
                            nc.gpsimd.partition_broadcast(zbc[:], rrow[:])
                            nc.vector.tensor_tensor(
                                out=xnT3[:, :, b * 128:(b + 1) * 128],
                                in0=xT3[:, :, b * 128:(b + 1) * 128],
                                in1=zbc[:].rearrange("p (k c) -> p k c", k=1)
                                    .broadcast_to((128, NKT, 128)),
                                op=mybir.AluOpType.mult)
                        return xnT_g

                    # Software-pipelined: norm chain of group g+1 is emitted
                    # BEFORE group g's K/V matmuls+evictions, so the FIFO
                    # Act/DVE queues never park the next norm behind
                    # PSUM-eviction waits. The local (Q/residual) path is
                    # deferred a couple of groups to keep the startup DMA
                    # device free for group 0/1 inputs.
                    NL0 = NG * 4
                    xnTs = {0: norm_group(xgs.pop(0), 0, pe_transpose=True,
                                          out_dtype=dt.float8e4)}
                    emit_w_dma(wv_all, WV)
                    xnq_holder = {}

                    def q_proj():
                        xnqT = xnq_holder.pop("q")
                        wq3 = wq_all[:].rearrange("p (kt c) -> p kt c", kt=NKT)
                        xnq3 = xnqT[:].rearrange("p (kt m) -> p kt m", kt=NKT)
                        for ob in range(NQB):
                            pq = ps_k.tile([128, SH], dt.float32, tag="pk", name="pq")
                            for pp in range(NKT // 2):
                                nc.tensor.matmul(pq[:],
                                                 wq3[:, 2 * pp:2 * pp + 2, ob * 128:(ob + 1) * 128],
                                                 xnq3[:, 2 * pp:2 * pp + 2, :],
                                                 start=(pp == 0), stop=(pp == NKT // 2 - 1),
                                                 perf_mode=mybir.MatmulPerfMode.DoubleRow)
                            nc.vector.tensor_copy(qT[0:96, ob * SH:(ob + 1) * SH], pq[0:96, :])

                    for g in range(NG):
                        if g + 2 < NG:
                            emit_xg_dma(g + 2)
                        if g == 0:
                            nc.sync.dma_start(x_res[:].rearrange("p (b c) -> p b c", b=NSB),
                                              Xs.ap().rearrange("(b p) c -> p b c", p=128))
                        if g + 1 < NG:
                            xnTs[g + 1] = norm_group(xgs.pop(g + 1), (g + 1) * 4,
                                                     pe_transpose=True,
                                                     out_dtype=dt.float8e4)
                        if g == 0:
                            emit_w_dma(wq_all, WQ)
                        if g == 1:
                            xnq_holder["q"] = norm_group(x_res, NL0, pe_transpose=True,
                                                         out_dtype=dt.float8e4)
                        xnT_g = xnTs.pop(g)
                        if g == 2:
                            q_proj()
                        # K^T for this group's 512 keys (fp8 DoubleRow:
                        # one matmul contracts a kt-PAIR = 256 dims)
                        wk3 = wk_all[:].rearrange("p (kt c) -> p kt c", kt=NKT)
                        xn3 = xnT_g[:].rearrange("p (kt m) -> p kt m", kt=NKT)
                        for ob in range(0 if _PROBE == "nokv" else NQB):
                            pk = ps_k.tile([128, SH], dt.float32, tag="pk", name="pk")
                            for pp in range(NKT // 2):
                                nc.tensor.matmul(pk[:],
                                                 wk3[:, 2 * pp:2 * pp + 2, ob * 128:(ob + 1) * 128],
                                                 xn3[:, 2 * pp:2 * pp + 2, :],
                                                 start=(pp == 0), stop=(pp == NKT // 2 - 1),
                                                 perf_mode=mybir.MatmulPerfMode.DoubleRow)
                            if ob % 2 == 0:
                                nc.scalar.copy(kT[0:96, ob * S + g * SH: ob * S + (g + 1) * SH], pk[0:96, :])
                            else:
                                nc.vector.tensor_copy(kT[0:96, ob * S + g * SH: ob * S + (g + 1) * SH], pk[0:96, :])
                        # V for this group's 4 key blocks (fp8 DoubleRow)
                        wv3 = wv_all[:].rearrange("p (kt c) -> p kt c", kt=NKT)
                        for kb in range(0 if _PROBE == "nokv" else 4):
                            kbg = g * 4 + kb
                            for jc in range(2):
                                pv = ps_v.tile([128, 384], dt.float32, tag="pv", name="pv")
                                for pp in range(NKT // 2):
                                    nc.tensor.matmul(pv[:],
                                                     xn3[:, 2 * pp:2 * pp + 2, kb * 128:(kb + 1) * 128],
                                                     wv3[:, 2 * pp:2 * pp + 2, jc * 384:(jc + 1) * 384],
                                                     start=(pp == 0), stop=(pp == NKT // 2 - 1),
                                                     perf_mode=mybir.MatmulPerfMode.DoubleRow)
                                dst = (v_full[:, kbg * VW + jc * 480: kbg * VW + (jc + 1) * 480]
                                       .rearrange("p (h c) -> p h c", c=HWID)[:, :, 0:64])
                                src = pv[:].rearrange("p (h c) -> p h c", c=64)
                                if jc == 0:
                                    nc.vector.tensor_copy(dst, src)
                                else:
                                    nc.scalar.copy(dst, src)

                # prefetch o_proj + MLP weights on the gpsimd SWDGE queue —
                # the SP queue already carries all 6.75MB of x loads, which
                # gate the phase-A norm chains, so another 7.1MB there would
                # make SP the phase-A critical path. SWDGE is otherwise idle.
                nc.gpsimd.dma_start(
                    wo_all[:].rearrange("p (kt c) -> p kt c", kt=NKT),
                    WO.ap().rearrange("(kt p) c -> p kt c", p=128))
                nc.gpsimd.dma_start(
                    wup_all[:].rearrange("p (kt c) -> p kt c", kt=NKT),
                    WU.ap().rearrange("(kt p) c -> p kt c", p=128))
                nc.gpsimd.dma_start(
                    wd_all[:].rearrange("p (it c) -> p it c", it=NIT),
                    WD.ap().rearrange("(it p) c -> p it c", p=128))

                # ============ phase C: attention ============
                with tc.tile_pool(name="att_e", bufs=8) as att_e, \
                     tc.tile_pool(name="att_w", bufs=2) as att_w, \
                     tc.tile_pool(name="sps_pool", bufs=3, space="PSUM") as sps_pool, \
                     tc.tile_pool(name="ops_pool", bufs=1, space="PSUM") as ops_pool:
                    k3s = kT[:].rearrange("p (g s) -> p g s", g=NQB)
                    q3s = qT[:].rearrange("p (g m) -> p g m", g=NQB)

                    def scores(t, kb):
                        s_ps = sps_pool.tile([128, 1024], dt.float32, tag="sps", name="s_ps")
                        for hh in range(2):
                            h = 2 * t + hh
                            tri, hs = divmod(h, 3)
                            nc.tensor.matmul(
                                s_ps[:, hh * SH:(hh + 1) * SH],
                                k3s[hs * 32:(hs + 1) * 32, 2 * tri:2 * tri + 2,
                                    kb * 128:(kb + 1) * 128],
                                q3s[hs * 32:(hs + 1) * 32, 2 * tri:2 * tri + 2, :],
                                start=True, stop=True,
                                perf_mode=mybir.MatmulPerfMode.DoubleRow)
                        return s_ps

                    # The exp stream is split 3 ways: ~3/8 on Act (native
                    # Exp) and ~5/8 as a DIRECT-to-int8 Schraudolph on DVE
                    # and Pool: the fp8e4m3 bit pattern of exp(x) is
                    # approximately int8(8*(x/log2 + 7) - 0.5), so ONE
                    # tensor_scalar (fp32 PSUM -> int8, written through an
                    # int8 bitcast of the fp8 eT) produces the fp8 exp.
                    # Per-element error ~2.6% mean / 8% max (numpy
                    # validated, same as the old int32-Schraudolph+cast
                    # path), washed out by softmax normalization. AV
                    # emission lags 2 pairs to hide the exp latency.
                    SCH8_A = float(8.0 / np.log(2.0) * SCALE / 4096.0)
                    SCH8_B = 55.5
                    AV_LAG = 3  # in kb-PAIR units

                    # AV runs as fp8 DoubleRow over kb-pairs: eT holds two
                    # consecutive key blocks' exp (fp8), one matmul per head
                    # contracts both (256 keys) at 0.5 cyc/row.
                    v3 = v_full[:].rearrange("p (kb c) -> p kb c", kb=NKB)

                    def emit_av(t, j, eT, o_pss):
                        e3 = eT[:].rearrange("p (two q) -> p two q", two=2)
                        for hh in range(2):
                            nc.tensor.matmul(
                                o_pss[hh][:],
                                v3[:, 2 * j:2 * j + 2, HWID * (2 * t + hh): HWID * (2 * t + hh) + HWID],
                                e3[:, :, hh * SH:(hh + 1) * SH],
                                start=(j == 0), stop=(j == NKB // 2 - 1),
                                perf_mode=mybir.MatmulPerfMode.DoubleRow)

                    if _PROBE == "noattn":
                        nc.gpsimd.memset(oT[:], 1.0)
                    s_next = None if _PROBE == "noattn" else scores(0, 0)
                    for t in range(0 if _PROBE == "noattn" else PAIRS):
                        o_ps0 = ops_pool.tile([HWID, SH], dt.float32, tag="o0", name="o_ps0")
                        o_ps1 = ops_pool.tile([HWID, SH], dt.float32, tag="o1", name="o_ps1")
                        o_pss = (o_ps0, o_ps1)
                        avq = []
                        eT_cur = None
                        for kb in range(NKB):
                            s_cur = s_next
                            if kb % 2 == 0:
                                eT_cur = att_e.tile([128, 2048], dt.float8e4, tag="eT", name="eT")
                            half = eT_cur[:, (kb % 2) * 1024:(kb % 2 + 1) * 1024]
                            if _PROBE == "noexp":
                                pass
                            else:
                                # Every block's exp is split column-wise
                                # across BOTH lanes at once (Act: native Exp
                                # on cols 0:576; DVE: direct-int8 Schraudolph
                                # on 576:1024 — Pool cannot read PSUM). Same
                                # aggregate engine load as alternating whole
                                # blocks, but each s_ps PSUM buffer frees in
                                # half the time (the recycle latency bounds
                                # the depth-3 pipeline), and each softmax
                                # column uses ONE exp method consistently so
                                # the Schraudolph bias cancels in the
                                # normalization. 576/448 matches the 1.2 /
                                # 0.96 GHz Act/DVE rates.
                                CSP = 576
                                nc.scalar.activation(half[:, 0:CSP],
                                                     s_cur[:, 0:CSP], AF.Exp,
                                                     scale=float(SCALE / 4096.0))
                                nc.vector.tensor_scalar(
                                    out=half[:, CSP:1024].bitcast(dt.int8),
                                    in0=s_cur[:, CSP:1024],
                                    scalar1=SCH8_A, scalar2=SCH8_B,
                                    op0=mybir.AluOpType.mult,
                                    op1=mybir.AluOpType.add)
                            if kb + 1 < NKB:
                                s_next = scores(t, kb + 1)
                            elif t + 1 < PAIRS:
                                s_next = scores(t + 1, 0)
                            if kb % 2 == 1:
                                avq.append((kb // 2, eT_cur))
                                if len(avq) > AV_LAG:
                                    emit_av(t, *avq.pop(0), o_pss)
                        for j_, eT_ in avq:
                            emit_av(t, j_, eT_, o_pss)
                        for hh in range(2):
                            b = 64 * hh
                            rz = att_w.tile([1, SH], dt.float32, tag="rz", name="rz")
                            nc.vector.reciprocal(rz[:], o_pss[hh][64:65, :])
                            zbc = att_w.tile([64, SH], dt.float32, tag="zbc", name="zbc")
                            nc.gpsimd.partition_broadcast(zbc[:], rz[:])
                            nc.vector.tensor_tensor(
                                out=oT[b:b + 64, t * SH:(t + 1) * SH],
                                in0=o_pss[hh][0:64, :], in1=zbc[:],
                                op=mybir.AluOpType.mult)

                # ===== phase D (still inside kvp): o_proj + rmsnorm2 =====
                xnT2 = tailp.tile([128, NKT * SH], dt.bfloat16)
                x2 = tailp.tile([128, NSB * D], dt.bfloat16)
                with tc.tile_pool(name="dworkp", bufs=2) as dworkp, \
                     tc.tile_pool(name="ps_d", bufs=2, space="PSUM") as ps_d:
                    oT3 = oT[:].rearrange("p (t m) -> p t m", t=PAIRS)
                    wo3 = wo_all[:].rearrange("p (kt c) -> p kt c", kt=NKT)
                    for sb in range(NSB):
                        for jc in range(2):
                            po = ps_d.tile([128, 384], dt.float32, tag="po", name="po")
                            for pp in range(NKT // 2):
                                nc.tensor.matmul(po[:],
                                                 oT3[:, 2 * pp:2 * pp + 2, sb * 128:(sb + 1) * 128],
                                                 wo3[:, 2 * pp:2 * pp + 2, jc * 384:(jc + 1) * 384],
                                                 start=(pp == 0), stop=(pp == NKT // 2 - 1),
                                                 perf_mode=mybir.MatmulPerfMode.DoubleRow)
                            nc.vector.scalar_tensor_tensor(
                                out=x2[:, sb * D + jc * 384: sb * D + (jc + 1) * 384],
                                in0=po[:], scalar=1.0 / 64.0,
                                in1=x_res[:, sb * D + jc * 384: sb * D + (jc + 1) * 384],
                                op0=mybir.AluOpType.mult, op1=mybir.AluOpType.add)
                        sq_accum(dworkp, x2[:, sb * D:(sb + 1) * D], NG * 4 + sb)
                    rr_for(NG * 4, 4, dworkp)
                    for sb in range(NSB):
                        xn2 = dworkp.tile([128, D], dt.bfloat16, tag="xn2", name="xn2")
                        nc.vector.tensor_scalar_mul(xn2[:], x2[:, sb * D:(sb + 1) * D],
                                                    rrs[:, NG * 4 + sb:NG * 4 + sb + 1])
                        # SP DMA queue is idle here (all weights prefetched),
                        # so transpose via DMA instead of burning PE cycles
                        xnT23 = xnT2[:].rearrange("p (kt m) -> p kt m", kt=NKT)
                        nc.sync.dma_start(
                            xnT23[:, :, sb * 128:(sb + 1) * 128], xn2[:],
                            transpose=True)

            # kv pool closed: kT / v_full / qT space reclaimed
            # ============ phase E: MLP ============
            if _PROBE == "nomlp":
                with tc.tile_pool(name="glob2", bufs=2) as glob2:
                    for sb in range(NSB):
                        o_sb = glob2.tile([128, D], dt.float32, tag="osb", name="o_sb")
                        nc.vector.tensor_copy(o_sb[:], x2[:, sb * D:(sb + 1) * D])
                        nc.sync.dma_start(OUT.ap()[sb * 128:(sb + 1) * 128, :], o_sb[:])
                return
            if True:
                # wup_all / wd_all already resident (prefetched end of phase A)
                with tc.tile_pool(name="mlpw", bufs=1) as mlpw, \
                     tc.tile_pool(name="eworkp", bufs=3) as eworkp, \
                     tc.tile_pool(name="ps_e", bufs=2, space="PSUM") as ps_e:
                    sgT = mlpw.tile([128, NIT * SH], dt.float8e4)
                    wu3 = wup_all[:].rearrange("p (kt c) -> p kt c", kt=NKT)
                    xn23 = xnT2[:].rearrange("p (kt m) -> p kt m", kt=NKT)
                    for it in range(NIT):
                        pu = ps_e.tile([128, SH], dt.float32, tag="pu", name="pu")
                        for kt in range(NKT):
                            nc.tensor.matmul(pu[:],
                                             wu3[:, kt, it * 128:(it + 1) * 128],
                                             xn23[:, kt, :],
                                             start=(kt == 0), stop=(kt == NKT - 1))
                        nc.scalar.activation(sgT[:, it * SH:(it + 1) * SH], pu[:], AF.Silu)
                    sg3 = sgT[:].rearrange("p (it m) -> p it m", it=NIT)
                    wd3 = wd_all[:].rearrange("p (it c) -> p it c", it=NIT)
                    for sb in range(NSB):
                        for jc in range(2):
                            pd = ps_e.tile([128, 384], dt.float32, tag="pd", name="pd")
                            for pp in range(NIT // 2):
                                nc.tensor.matmul(pd[:],
                                                 sg3[:, 2 * pp:2 * pp + 2, sb * 128:(sb + 1) * 128],
                                                 wd3[:, 2 * pp:2 * pp + 2, jc * 384:(jc + 1) * 384],
                                                 start=(pp == 0), stop=(pp == NIT // 2 - 1),
                                                 perf_mode=mybir.MatmulPerfMode.DoubleRow)
                            o_sb = eworkp.tile([128, 384], dt.float32, tag="osb", name="o_sb")
                            nc.vector.scalar_tensor_tensor(
                                out=o_sb[:], in0=pd[:], scalar=1.0 / 64.0,
                                in1=x2[:, sb * D + jc * 384: sb * D + (jc + 1) * 384],
                                op0=mybir.AluOpType.mult, op1=mybir.AluOpType.add)
                            nc.sync.dma_start(OUT.ap()[sb * 128:(sb + 1) * 128, jc * 384:(jc + 1) * 384],
                                              o_sb[:])


def _get_nc(reps=1):
    key = ("nc", reps)
    if key not in _CACHED:
        _CACHED[key] = _build(reps)
    return _CACHED[key]


def _qk_perm():
    """Output-dim order for wq/wk columns that lands K^T/Q^T directly in
    the DoubleRow scores layout: column group g = (tri, half) holds, for
    the 3 heads of triple tri, dims [64*head + 32*half : +32] stacked
    3x32 across partitions 0..96 (cols 96..128 of each group are zero
    padding). Returns (index, valid-mask), both length DP."""
    idx = np.zeros(DP, np.int64)
    valid = np.zeros(DP, bool)
    for g in range(NQB):
        tri, half = divmod(g, 2)
        for hs in range(3):
            head = tri * 3 + hs
            base = 64 * head + 32 * half
            s0 = g * 128 + hs * 32
            idx[s0:s0 + 32] = np.arange(base, base + 32)
            valid[s0:s0 + 32] = True
    return idx, valid


def _prep_in_maps(hidden_states, wq, wk, wv, wo, w_up, w_down, ln1_w, ln2_w):
    bf16 = ml_dtypes.bfloat16
    x = np.asarray(hidden_states, np.float32).reshape(S, D)
    x_bf = np.ascontiguousarray(x.astype(bf16))
    idx, valid = _qk_perm()
    fp8 = mybir.dt.np(mybir.dt.float8e4)
    wqT = np.ascontiguousarray(
        ((np.asarray(wq, np.float32) * np.asarray(ln1_w, np.float32)[None, :] * 64.0).T[:, idx]
         * valid[None, :]).astype(fp8))
    wkT = np.ascontiguousarray(
        ((np.asarray(wk, np.float32) * np.asarray(ln1_w, np.float32)[None, :] * 64.0).T[:, idx]
         * valid[None, :]).astype(fp8))
    wvT = np.ascontiguousarray((np.asarray(wv, np.float32) * np.asarray(ln1_w, np.float32)[None, :] * 64.0).T.astype(fp8))
    woT = np.ascontiguousarray((np.asarray(wo, np.float32) * 64.0).T.astype(fp8))
    wupT = np.ascontiguousarray((np.asarray(w_up, np.float32) * np.asarray(ln2_w, np.float32)[None, :]).T.astype(bf16))
    wdownT = np.ascontiguousarray((np.asarray(w_down, np.float32) * 64.0).T.astype(fp8))
    return [{
        "x_shard": np.ascontiguousarray(x[c * SH:(c + 1) * SH].astype(bf16)),
        "x_full": x_bf,
        "wqT": wqT, "wkT": wkT, "wvT": wvT, "woT": woT,
        "wupT": wupT, "wdownT": wdownT,
    } for c in range(N_CORES)]


def _get_runner(reps=1):
    """Build the sharded jitted executable once; reuse across calls."""
    if ("runner", reps) in _CACHED:
        return _CACHED[("runner", reps)]
    import jax
    from jax.sharding import Mesh, PartitionSpec
    try:
        from jax.experimental.shard_map import shard_map
    except ImportError:
        shard_map = jax.shard_map
    from concourse import bass2jax
    bass2jax.install_neuronx_cc_hook()
    nc = _get_nc(reps)
    import concourse.mybir as mybir_m
    partition_name = nc.partition_id_tensor.name if nc.partition_id_tensor else None
    in_names, out_names, out_avals, zero_outs = [], [], [], []
    for alloc in nc.m.functions[0].allocations:
        if not isinstance(alloc, mybir_m.MemoryLocationSet):
            continue
        name = alloc.memorylocations[0].name
        if alloc.kind == "ExternalInput":
            if name != partition_name:
                in_names.append(name)
        elif alloc.kind == "ExternalOutput":
            out_names.append(name)
            shape = tuple(alloc.tensor_shape)
            dtype = mybir_m.dt.np(alloc.dtype)
            out_avals.append(jax.core.ShapedArray(shape, dtype))
            zero_outs.append(np.zeros(shape, dtype))
    n_params = len(in_names)
    all_names = list(in_names) + list(out_names)
    if partition_name is not None:
        all_names.append(partition_name)

    def _body(*args):
        operands = list(args)
        if partition_name is not None:
            operands.append(bass2jax.partition_id_tensor())
        outs = bass2jax._bass_exec_p.bind(
            *operands, out_avals=tuple(out_avals), in_names=tuple(all_names),
            out_names=tuple(out_names), lowering_input_output_aliases=(),
            sim_require_finite=True, sim_require_nnan=True, nc=nc)
        return tuple(outs)

    devices = jax.devices()[:N_CORES]
    mesh = Mesh(np.asarray(devices), ("core",))
    in_specs = (PartitionSpec("core"),) * (n_params + len(out_names))
    out_specs = (PartitionSpec("core"),) * len(out_names)
    fn = jax.jit(shard_map(_body, mesh=mesh, in_specs=in_specs,
                           out_specs=out_specs, check_rep=False))

    def run(in_maps):
        concat_in = [np.concatenate([np.asarray(in_maps[c][n]) for c in range(N_CORES)], axis=0)
                     for n in in_names]
        concat_zero = [np.zeros((N_CORES * z.shape[0], *z.shape[1:]), z.dtype) for z in zero_outs]
        out_arrs = fn(*concat_in, *concat_zero)
        jax.block_until_ready(out_arrs)
        return {name: np.asarray(out_arrs[i]) for i, name in enumerate(out_names)}

    if reps == 1:
        _CACHED["runner_parts"] = (fn, in_names, out_names, zero_outs, mesh)
        _CACHED["runner"] = run
    _CACHED[("runner", reps)] = run
    _CACHED[("runner_fn", reps)] = fn
    return run


def _get_repeat_runner(reps):
    """Jitted executable whose bass NEFF runs the full kernel body `reps`
    times back-to-back on device (the body is emitted `reps` times into one
    program; every DMA/compute genuinely re-executes). Used by test.py to
    measure per-execution device time without per-dispatch tunnel/client
    overhead."""
    _get_runner(reps)
    return _CACHED[("runner_fn", reps)]


def kernel(hidden_states, wq, wk, wv, wo, w_up, w_down, ln1_w, ln2_w):
    in_maps = _prep_in_maps(hidden_states, wq, wk, wv, wo, w_up, w_down, ln1_w, ln2_w)
    try:
        run = _get_runner()
        outs = run(in_maps)
        out = outs["out"].reshape(N_CORES, SH, D).reshape(S, D)
    except Exception:
        nc = _get_nc()
        res = run_bass_kernel_spmd(nc, in_maps, core_ids=list(range(N_CORES)))
        out = np.concatenate([res.results[c]["out"] for c in range(N_CORES)], axis=0)
    return out.reshape(1, S, D).astype(np.float32)

